# revision 4
# baseline (speedup 1.0000x reference)
"""PhaseEncoding kernel for Trainium2 (8 NeuronCores, SPMD).

Computes out = x + einsum('sbp,pd->sbd', phase_one_hot, emb_table)
with x:(4096,8,1024) f32, phase_one_hot:(4096,8,9) f32, emb_table:(9,1024) f32.

Sharding: seq dim (4096) split 8 ways -> per core 512*8=4096 tokens.
Per-core Bass kernel:
  - phase_T (9, 4096) and emb (9, 1024) resident in SBUF
  - stream x in 2 MiB chunks ([128 part, 4, 1024])
  - per 128-token block: PE matmul out[tok,d] = sum_p phase_T[p,tok]*emb[p,d]
    into PSUM, DVE add with x, DMA result out.
"""

import numpy as np

import concourse.bacc as bacc
import concourse.bass as bass
import concourse.tile as tile
from concourse import mybir
from concourse.bass_utils import run_bass_kernel_spmd

# Full-problem shapes (hardcoded per contract).
S, B, D, P = 4096, 8, 1024, 9
N_CORES = 8
S_LOC = S // N_CORES          # 512 seq positions per core
TOK = S_LOC * B               # 4096 tokens per core

F32 = mybir.dt.float32


def build_program(tok=TOK, d=D, p=P, blocks_per_chunk=4):
    """Build the per-core Bass program. Returns the Bass object."""
    assert tok % 128 == 0
    n_blocks = tok // 128
    a = blocks_per_chunk
    assert n_blocks % a == 0
    n_chunks = n_blocks // a
    n_halves = d // 512

    nc = bacc.Bacc("TRN2", debug=False, target_bir_lowering=False)

    x_dram = nc.dram_tensor("x", [tok, d], F32, kind="ExternalInput")
    pt_dram = nc.dram_tensor("phase_t", [p, tok], F32, kind="ExternalInput")
    emb_dram = nc.dram_tensor("emb", [p, d], F32, kind="ExternalInput")
    out_dram = nc.dram_tensor("out", [tok, d], F32, kind="ExternalOutput")

    with tile.TileContext(nc) as tc:
        with (
            tc.tile_pool(name="const", bufs=1) as cpool,
            tc.tile_pool(name="xin", bufs=3) as inpool,
            tc.tile_pool(name="xout", bufs=3) as outpool,
            tc.tile_pool(name="acc", bufs=4, space="PSUM") as psumpool,
        ):
            pt_sb = cpool.tile([p, tok], F32)
            nc.sync.dma_start(pt_sb[:], pt_dram.ap())
            emb_sb = cpool.tile([p, d], F32)
            nc.sync.dma_start(emb_sb[:], emb_dram.ap())

            x_view = x_dram.ap().rearrange("(c a q) d -> c q a d", a=a, q=128)
            o_view = out_dram.ap().rearrange("(c a q) d -> c q a d", a=a, q=128)

            for c in range(n_chunks):
                xt = inpool.tile([128, a, d], F32)
                nc.sync.dma_start(xt[:], x_view[c])
                ot = outpool.tile([128, a, d], F32)
                for ai in range(a):
                    blk = c * a + ai
                    ps = psumpool.tile([128, d], F32)
                    for n in range(n_halves):
                        nc.tensor.matmul(
                            ps[:, bass.ts(n, 512)],
                            pt_sb[:, bass.ts(blk, 128)],
                            emb_sb[:, bass.ts(n, 512)],
                            start=True,
                            stop=True,
                        )
                    nc.vector.tensor_add(ot[:, ai, :], xt[:, ai, :], ps[:])
                nc.scalar.dma_start(o_view[c], ot[:])

    nc.finalize()
    return nc


_NC = None


def _get_nc():
    global _NC
    if _NC is None:
        _NC = build_program()
    return _NC


def make_in_maps(x, phase_one_hot, emb_table):
    x = np.ascontiguousarray(np.asarray(x, dtype=np.float32))
    ph = np.ascontiguousarray(np.asarray(phase_one_hot, dtype=np.float32))
    emb = np.ascontiguousarray(np.asarray(emb_table, dtype=np.float32))
    in_maps = []
    for c in range(N_CORES):
        xs = x[c * S_LOC : (c + 1) * S_LOC].reshape(TOK, D)
        ps = np.ascontiguousarray(
            ph[c * S_LOC : (c + 1) * S_LOC].reshape(TOK, P).T
        )
        in_maps.append({"x": xs, "phase_t": ps, "emb": emb})
    return in_maps


def run_sharded(in_maps, trace=False, **kwargs):
    nc = _get_nc()
    return run_bass_kernel_spmd(nc, in_maps, list(range(N_CORES)), trace=trace, **kwargs)


def kernel(x, phase_one_hot, emb_table):
    in_maps = make_in_maps(x, phase_one_hot, emb_table)
    res = run_sharded(in_maps)
    out = np.concatenate(
        [r["out"].reshape(S_LOC, B, D) for r in res.results], axis=0
    )
    return out


# revision 5
# speedup vs baseline: 1.3705x; 1.3705x over previous
"""PhaseEncoding kernel for Trainium2 (8 NeuronCores, SPMD).

Computes out = x + einsum('sbp,pd->sbd', phase_one_hot, emb_table)
with x:(4096,8,1024) f32, phase_one_hot:(4096,8,9) f32, emb_table:(9,1024) f32.

Sharding: seq dim (4096) split 8 ways -> per core 512*8=4096 tokens.

PE trick: fp32 matmul runs at 1/4 rate, so split phase and emb into
bf16 hi+lo parts and stack all four cross products along the K
(contraction) axis: K=36 <= 128 partitions, so one bf16-rate matmul
computes the full compensated product (error ~1e-5 vs fp32).
"""

import numpy as np
import ml_dtypes

import concourse.bacc as bacc
import concourse.bass as bass
import concourse.tile as tile
from concourse import mybir
from concourse.bass_utils import run_bass_kernel_spmd

# Full-problem shapes (hardcoded per contract).
S, B, D, P = 4096, 8, 1024, 9
N_CORES = 8
S_LOC = S // N_CORES          # 512 seq positions per core
TOK = S_LOC * B               # 4096 tokens per core
K = 4 * P                     # hi/lo cross-product stack

F32 = mybir.dt.float32
BF16 = mybir.dt.bfloat16
NP_BF16 = ml_dtypes.bfloat16


def build_program(tok=TOK, d=D, k=K, blocks_per_chunk=4):
    """Build the per-core Bass program. Returns the Bass object."""
    assert tok % 128 == 0
    n_blocks = tok // 128
    a = blocks_per_chunk
    assert n_blocks % a == 0
    n_chunks = n_blocks // a
    n_halves = d // 512

    nc = bacc.Bacc("TRN2", debug=False, target_bir_lowering=False)

    x_dram = nc.dram_tensor("x", [tok, d], F32, kind="ExternalInput")
    pt_dram = nc.dram_tensor("phase_t", [k, tok], BF16, kind="ExternalInput")
    emb_dram = nc.dram_tensor("emb", [k, d], BF16, kind="ExternalInput")
    out_dram = nc.dram_tensor("out", [tok, d], F32, kind="ExternalOutput")

    with tile.TileContext(nc) as tc:
        with (
            tc.tile_pool(name="const", bufs=1) as cpool,
            tc.tile_pool(name="xin", bufs=3) as inpool,
            tc.tile_pool(name="xout", bufs=3) as outpool,
            tc.tile_pool(name="acc", bufs=4, space="PSUM") as psumpool,
        ):
            pt_sb = cpool.tile([k, tok], BF16)
            nc.sync.dma_start(pt_sb[:], pt_dram.ap())
            emb_sb = cpool.tile([k, d], BF16)
            nc.sync.dma_start(emb_sb[:], emb_dram.ap())

            x_view = x_dram.ap().rearrange("(c a q) d -> c q a d", a=a, q=128)
            o_view = out_dram.ap().rearrange("(c a q) d -> c q a d", a=a, q=128)

            for c in range(n_chunks):
                xt = inpool.tile([128, a, d], F32)
                nc.sync.dma_start(xt[:], x_view[c])
                ot = outpool.tile([128, a, d], F32)
                for ai in range(a):
                    blk = c * a + ai
                    ps = psumpool.tile([128, d], F32)
                    for n in range(n_halves):
                        nc.tensor.matmul(
                            ps[:, bass.ts(n, 512)],
                            pt_sb[:, bass.ts(blk, 128)],
                            emb_sb[:, bass.ts(n, 512)],
                            start=True,
                            stop=True,
                        )
                    nc.vector.tensor_add(ot[:, ai, :], xt[:, ai, :], ps[:])
                nc.scalar.dma_start(o_view[c], ot[:])

    nc.finalize()
    return nc


_NC = None


def _get_nc():
    global _NC
    if _NC is None:
        _NC = build_program()
    return _NC


def _hi_lo(arr):
    hi = arr.astype(NP_BF16)
    lo = (arr - hi.astype(np.float32)).astype(NP_BF16)
    return hi, lo


def make_in_maps(x, phase_one_hot, emb_table):
    x = np.ascontiguousarray(np.asarray(x, dtype=np.float32))
    ph = np.asarray(phase_one_hot, dtype=np.float32)
    emb = np.asarray(emb_table, dtype=np.float32)

    ehi, elo = _hi_lo(emb)
    emb_stack = np.ascontiguousarray(
        np.concatenate([ehi, elo, ehi, elo], axis=0)
    )

    in_maps = []
    for c in range(N_CORES):
        xs = x[c * S_LOC : (c + 1) * S_LOC].reshape(TOK, D)
        pt = ph[c * S_LOC : (c + 1) * S_LOC].reshape(TOK, P).T
        phi, plo = _hi_lo(pt)
        pt_stack = np.ascontiguousarray(
            np.concatenate([phi, phi, plo, plo], axis=0)
        )
        in_maps.append({"x": xs, "phase_t": pt_stack, "emb": emb_stack})
    return in_maps


def run_sharded(in_maps, trace=False, **kwargs):
    nc = _get_nc()
    return run_bass_kernel_spmd(nc, in_maps, list(range(N_CORES)), trace=trace, **kwargs)


def kernel(x, phase_one_hot, emb_table):
    in_maps = make_in_maps(x, phase_one_hot, emb_table)
    res = run_sharded(in_maps)
    out = np.concatenate(
        [r["out"].reshape(S_LOC, B, D) for r in res.results], axis=0
    )
    return out


# revision 6
# speedup vs baseline: 1.3766x; 1.0044x over previous
"""PhaseEncoding kernel for Trainium2 (8 NeuronCores, SPMD).

Computes out = x + einsum('sbp,pd->sbd', phase_one_hot, emb_table)
with x:(4096,8,1024) f32, phase_one_hot:(4096,8,9) f32, emb_table:(9,1024) f32.

Sharding: seq dim (4096) split 8 ways -> per core 512*8=4096 tokens.

PE trick: fp32 matmul runs at 1/4 rate, so split phase and emb into
bf16 hi+lo parts and stack all four cross products along the K
(contraction) axis: K=36 <= 128 partitions, so one bf16-rate matmul
computes the full compensated product (error ~1e-5 vs fp32).
"""

import numpy as np
import ml_dtypes

import concourse.bacc as bacc
import concourse.bass as bass
import concourse.tile as tile
from concourse import mybir
from concourse.bass_utils import run_bass_kernel_spmd

# Full-problem shapes (hardcoded per contract).
S, B, D, P = 4096, 8, 1024, 9
N_CORES = 8
S_LOC = S // N_CORES          # 512 seq positions per core
TOK = S_LOC * B               # 4096 tokens per core
K = 4 * P                     # hi/lo cross-product stack

F32 = mybir.dt.float32
BF16 = mybir.dt.bfloat16
NP_BF16 = ml_dtypes.bfloat16


def build_program(tok=TOK, d=D, k=K, blocks_per_chunk=2, bufs=8):
    """Build the per-core Bass program. Returns the Bass object."""
    assert tok % 128 == 0
    n_blocks = tok // 128
    a = blocks_per_chunk
    assert n_blocks % a == 0
    n_chunks = n_blocks // a
    n_halves = d // 512

    nc = bacc.Bacc("TRN2", debug=False, target_bir_lowering=False)

    x_dram = nc.dram_tensor("x", [tok, d], F32, kind="ExternalInput")
    pt_dram = nc.dram_tensor("phase_t", [k, tok], BF16, kind="ExternalInput")
    emb_dram = nc.dram_tensor("emb", [k, d], BF16, kind="ExternalInput")
    out_dram = nc.dram_tensor("out", [tok, d], F32, kind="ExternalOutput")

    with tile.TileContext(nc) as tc:
        with (
            tc.tile_pool(name="const", bufs=1) as cpool,
            tc.tile_pool(name="xin", bufs=bufs) as inpool,
            tc.tile_pool(name="acc", bufs=4, space="PSUM") as psumpool,
        ):
            pt_sb = cpool.tile([k, tok], BF16)
            nc.sync.dma_start(pt_sb[:], pt_dram.ap())
            emb_sb = cpool.tile([k, d], BF16)
            nc.sync.dma_start(emb_sb[:], emb_dram.ap())

            x_view = x_dram.ap().rearrange("(c a q) d -> c q a d", a=a, q=128)
            o_view = out_dram.ap().rearrange("(c a q) d -> c q a d", a=a, q=128)

            for c in range(n_chunks):
                xt = inpool.tile([128, a, d], F32)
                nc.sync.dma_start(xt[:], x_view[c])
                for ai in range(a):
                    blk = c * a + ai
                    ps = psumpool.tile([128, d], F32)
                    for n in range(n_halves):
                        nc.tensor.matmul(
                            ps[:, bass.ts(n, 512)],
                            pt_sb[:, bass.ts(blk, 128)],
                            emb_sb[:, bass.ts(n, 512)],
                            start=True,
                            stop=True,
                        )
                    nc.vector.tensor_add(xt[:, ai, :], xt[:, ai, :], ps[:])
                nc.scalar.dma_start(o_view[c], xt[:])

    nc.finalize()
    return nc


_NC = None


def _get_nc():
    global _NC
    if _NC is None:
        _NC = build_program()
    return _NC


def _hi_lo(arr):
    hi = arr.astype(NP_BF16)
    lo = (arr - hi.astype(np.float32)).astype(NP_BF16)
    return hi, lo


def make_in_maps(x, phase_one_hot, emb_table):
    x = np.ascontiguousarray(np.asarray(x, dtype=np.float32))
    ph = np.asarray(phase_one_hot, dtype=np.float32)
    emb = np.asarray(emb_table, dtype=np.float32)

    ehi, elo = _hi_lo(emb)
    emb_stack = np.ascontiguousarray(
        np.concatenate([ehi, elo, ehi, elo], axis=0)
    )

    in_maps = []
    for c in range(N_CORES):
        xs = x[c * S_LOC : (c + 1) * S_LOC].reshape(TOK, D)
        pt = ph[c * S_LOC : (c + 1) * S_LOC].reshape(TOK, P).T
        phi, plo = _hi_lo(pt)
        pt_stack = np.ascontiguousarray(
            np.concatenate([phi, phi, plo, plo], axis=0)
        )
        in_maps.append({"x": xs, "phase_t": pt_stack, "emb": emb_stack})
    return in_maps


def run_sharded(in_maps, trace=False, **kwargs):
    nc = _get_nc()
    return run_bass_kernel_spmd(nc, in_maps, list(range(N_CORES)), trace=trace, **kwargs)


def kernel(x, phase_one_hot, emb_table):
    in_maps = make_in_maps(x, phase_one_hot, emb_table)
    res = run_sharded(in_maps)
    out = np.concatenate(
        [r["out"].reshape(S_LOC, B, D) for r in res.results], axis=0
    )
    return out
